# revision 1
# baseline (speedup 1.0000x reference)
"""AttentionBlock (GroupNorm -> qkv -> MHA -> proj -> residual) on 8 trn2 cores.

Data-parallel over batch: 16 batches -> 2 per core. No collectives.

Per-core math (per batch item, c=512 channels, hw=1024 spatial, 8 heads x 64):
  xn = groupnorm(x)                     [c, hw] layout (c on partitions)
  q,k = Wqk^T.T @ xn + b                [2c, hw]
  vT  = xn.T @ WvT + bv (broadcast)     [hw, c]   (direct transposed matmul!)
  per head: S^T = k^T q                 [s=hw, t=hw]   (d=64 contraction)
            P = exp(S^T / 8)            (softmax w/o max-sub; logits are ~N(0,1))
            AV: lhsT=[vT_h | ones] -> rows 0..64 unnormalized out, row 64 = rowsum r
            h = AV[0:64] * (1/r)        (K=1 broadcast matmul into rows 64:128 + DVE mul)
  y = x + WprojT.T @ h + proj_b

All big matmuls run as float32r (full PE rate for moving dim >= 256, fp32 bits).
GroupNorm rstd = exp(-0.5*ln(var+eps)) so ACT only ever needs one table set.
"""

import os

import numpy as np
import ml_dtypes

import concourse.bass as bass
import concourse.tile as tile
import concourse.mybir as mybir
from concourse import bacc
from concourse.masks import make_identity

NUM_HEADS = 8
NUM_GROUPS = 32
EPS = 1e-5
B, C, H, W = 16, 512, 32, 32
HW = H * W                  # 1024
NCORES = 8
BPC = B // NCORES           # 2 batches per core
HD = C // NUM_HEADS         # 64
GS = C // NUM_GROUPS        # 16 channels per group
CT = C // 128               # 4 channel tiles
QKT = 2 * C // 128          # 8 q+k output tiles
ST = HW // 128              # 8 sequence tiles
NH = HW // 512              # 2 moving-dim chunks of 512

F32 = mybir.dt.float32
F32R = mybir.dt.float32r
BF16 = mybir.dt.bfloat16
ALU = mybir.AluOpType
ACTF = mybir.ActivationFunctionType
USE_TP = os.environ.get("KERNEL_NO_TP") != "1"


def _r(ap):
    return ap.bitcast(F32R)


def build(num_devices=NCORES):
    nc = bacc.Bacc("TRN2", target_bir_lowering=False, debug=False,
                   num_devices=num_devices)

    x_d = nc.dram_tensor("x", [BPC, C, HW], F32, kind="ExternalInput").ap()
    wqkvT_d = nc.dram_tensor("wqkvT", [C, 3 * C], BF16, kind="ExternalInput").ap()
    qkvb_d = nc.dram_tensor("qkvb", [3 * C], F32, kind="ExternalInput").ap()
    wprojT_d = nc.dram_tensor("wprojT", [C, C], BF16, kind="ExternalInput").ap()
    projb_d = nc.dram_tensor("projb", [C], F32, kind="ExternalInput").ap()
    gng_d = nc.dram_tensor("gng", [C], F32, kind="ExternalInput").ap()
    gnb_d = nc.dram_tensor("gnb", [C], F32, kind="ExternalInput").ap()
    # gmat[t] : [128, 32] with [k, g] = 1 iff (128t+k)//16 == g  (group reduce)
    gmat_d = nc.dram_tensor("gmat", [CT, 128, NUM_GROUPS], F32, kind="ExternalInput").ap()
    # emat[t] : [32, 128] transpose of gmat[t]  (group -> channel expand)
    emat_d = nc.dram_tensor("emat", [CT, NUM_GROUPS, 128], F32, kind="ExternalInput").ap()
    out_d = nc.dram_tensor("out", [BPC, C, HW], F32, kind="ExternalOutput").ap()

    with tile.TileContext(nc) as tc:
        _body(tc, nc, x_d, wqkvT_d, qkvb_d, wprojT_d, projb_d, gng_d, gnb_d,
              gmat_d, emat_d, out_d)
    nc.compile()
    return nc


def _body(tc, nc, x_d, wqkvT_d, qkvb_d, wprojT_d, projb_d, gng_d, gnb_d,
          gmat_d, emat_d, out_d):
    from contextlib import ExitStack
    ctx = ExitStack()
    with ctx:
        const = ctx.enter_context(tc.tile_pool(name="const", bufs=1))
        xpool = ctx.enter_context(tc.tile_pool(name="xpool", bufs=2 * CT))
        xnpool = ctx.enter_context(tc.tile_pool(name="xnpool", bufs=2 * CT))
        qkvpool = ctx.enter_context(tc.tile_pool(name="qkvpool", bufs=2 * QKT))
        vtapool = ctx.enter_context(tc.tile_pool(name="vtapool", bufs=2 * ST))
        exppool = ctx.enter_context(tc.tile_pool(name="exppool", bufs=6))
        hpool = ctx.enter_context(tc.tile_pool(name="hpool", bufs=2 * CT))
        ypool = ctx.enter_context(tc.tile_pool(name="ypool", bufs=2))
        smalls = ctx.enter_context(tc.tile_pool(name="smalls", bufs=6))
        rpool = ctx.enter_context(tc.tile_pool(name="rpool", bufs=2))
        hupool = ctx.enter_context(tc.tile_pool(name="hupool", bufs=3))
        
        rbpool = ctx.enter_context(tc.tile_pool(name="rbpool", bufs=2))
        drams = ctx.enter_context(tc.tile_pool(name="drams", bufs=2, space="DRAM"))
        ps_big = ctx.enter_context(tc.tile_pool(name="ps_big", bufs=2, space="PSUM"))
        ps_av = ctx.enter_context(tc.tile_pool(name="ps_av", bufs=2, space="PSUM"))

        # ---- constants (small ones first so x DMAs + GN start instantly;
        #      the big weight DMAs are emitted after emit_gn(0) below) ----
        wq_sb = const.tile([128, CT, 3 * C], BF16)      # wqkvT, 4 x [128, 1536]
        wp_sb = const.tile([128, CT, C], BF16)          # wprojT, 4 x [128, 512]
        gm_sb = const.tile([128, CT, NUM_GROUPS], F32)
        for t in range(CT):
            nc.sync.dma_start(out=gm_sb[:, t, :], in_=gmat_d[t])
        em_sb = const.tile([NUM_GROUPS, CT, 128], F32)
        for t in range(CT):
            nc.sync.dma_start(out=em_sb[:, t, :], in_=emat_d[t])
        qkvb_sb = const.tile([128, QKT], F32)
        for t in range(QKT):
            nc.sync.dma_start(out=qkvb_sb[:, t:t + 1], in_=qkvb_d[t * 128:(t + 1) * 128][:, None])
        # v-bias broadcast to all partitions: [128, 8, 64] (channel 2C+64h+d at [., h, d])
        vbias_bc = const.tile([128, NUM_HEADS, HD], F32)
        vb_src = qkvb_d[2 * C:3 * C]
        vb_bcast = bass.AP(tensor=vb_src.tensor, offset=vb_src.offset,
                           ap=[[0, 128]] + list(vb_src.ap))
        nc.sync.dma_start(out=vbias_bc,
                          in_=vb_bcast.rearrange("p (h d) -> p h d", h=NUM_HEADS))
        projb_sb = const.tile([128, CT], F32)
        gng_sb = const.tile([128, CT], F32)
        gnb_sb = const.tile([128, CT], F32)
        for t in range(CT):
            nc.sync.dma_start(out=projb_sb[:, t:t + 1], in_=projb_d[t * 128:(t + 1) * 128][:, None])
            nc.sync.dma_start(out=gng_sb[:, t:t + 1], in_=gng_d[t * 128:(t + 1) * 128][:, None])
            nc.sync.dma_start(out=gnb_sb[:, t:t + 1], in_=gnb_d[t * 128:(t + 1) * 128][:, None])
        magic = const.tile([NUM_GROUPS, 1], mybir.dt.uint32)
        nc.vector.memset(magic, 0x5F3759DF)

        def emit_weight_loads():
            for kt in range(CT):
                nc.sync.dma_start(out=wq_sb[:, kt, :], in_=wqkvT_d[kt * 128:(kt + 1) * 128, :])
            for kt in range(CT):
                nc.sync.dma_start(out=wp_sb[:, kt, :], in_=wprojT_d[kt * 128:(kt + 1) * 128, :])

        state = [dict() for _ in range(BPC)]

        def emit_gn(b):
            """x DMA + group-norm -> xts, xns."""
            s = state[b]
            xts, cm2s, xns = [], [], []
            for ct in range(CT):
                xt = xpool.tile([128, HW], F32, tag="xt", name=f"xt_{b}_{ct}")
                for sg in range(2):
                    nc.sync.dma_start(out=xt[:, sg * 512:(sg + 1) * 512],
                                      in_=x_d[b, ct * 128:(ct + 1) * 128, sg * 512:(sg + 1) * 512])
                xts.append(xt)
                stats = smalls.tile([128, 2, 6], F32, tag="bnst", name=f"bnst_{b}_{ct}")
                for sg in range(2):
                    nc.vector.bn_stats(out=stats[:, sg, :], in_=xt[:, sg * 512:(sg + 1) * 512])
                cmv = smalls.tile([128, 2], F32, tag="cmv", name=f"cmv_{b}_{ct}")
                nc.vector.bn_aggr(out=cmv, in_=stats)
                # cm2 = [mean_c, E[x^2]_c] ; E[x^2] = var + mean^2
                cm2 = smalls.tile([128, 2], F32, tag="cm2", name=f"cm2_{b}_{ct}")
                nc.vector.tensor_copy(out=cm2[:, 0:1], in_=cmv[:, 0:1])
                nc.vector.tensor_tensor(out=cm2[:, 1:2], in0=cmv[:, 0:1], in1=cmv[:, 0:1], op=ALU.mult)
                nc.vector.tensor_tensor(out=cm2[:, 1:2], in0=cm2[:, 1:2], in1=cmv[:, 1:2], op=ALU.add)
                cm2s.append(cm2)
                xn = xnpool.tile([128, HW], BF16, tag="xn", name=f"xn_{b}_{ct}")
                xns.append(xn)
            ps_g = ps_big.tile([128, HW], F32, tag="psbig", name=f"psg_{b}")
            for ct in range(CT):
                nc.tensor.matmul(ps_g[0:NUM_GROUPS, 0:2], lhsT=gm_sb[:, ct, :], rhs=cm2s[ct],
                                 start=(ct == 0), stop=(ct == CT - 1))
            # group stats: gstat = [mean_g, rstd_g]; rstd = rsqrt(var+eps)
            # computed fully on DVE (quake seed + 3 Newton steps) so the
            # ScalarE only ever needs the exp table set (no set swaps).
            gstat = smalls.tile([NUM_GROUPS, 2], F32, tag="gstat", name=f"gstat_{b}")
            nc.vector.tensor_scalar_mul(out=gstat, in0=ps_g[0:NUM_GROUPS, 0:2], scalar1=1.0 / GS)
            var_g = smalls.tile([NUM_GROUPS, 1], F32, tag="varg", name=f"varg_{b}")
            nc.vector.tensor_tensor(out=var_g, in0=gstat[:, 0:1], in1=gstat[:, 0:1], op=ALU.mult)
            nc.vector.tensor_tensor(out=var_g, in0=gstat[:, 1:2], in1=var_g, op=ALU.subtract)
            nc.vector.tensor_scalar_add(out=var_g, in0=var_g, scalar1=EPS)
            y_n = smalls.tile([NUM_GROUPS, 1], F32, tag="yn", name=f"yn_{b}")
            t_n = smalls.tile([NUM_GROUPS, 1], F32, tag="tn", name=f"tn_{b}")
            nc.vector.tensor_scalar(out=y_n.bitcast(mybir.dt.uint32),
                                    in0=var_g.bitcast(mybir.dt.uint32),
                                    scalar1=1, scalar2=None, op0=ALU.logical_shift_right)
            nc.vector.tensor_tensor(out=y_n.bitcast(mybir.dt.uint32), in0=magic,
                                    in1=y_n.bitcast(mybir.dt.uint32), op=ALU.subtract)
            for _ in range(3):
                nc.vector.tensor_tensor(out=t_n, in0=var_g, in1=y_n, op=ALU.mult)
                nc.vector.tensor_tensor(out=t_n, in0=t_n, in1=y_n, op=ALU.mult)
                nc.vector.tensor_scalar(out=t_n, in0=t_n, scalar1=-0.5, scalar2=1.5,
                                        op0=ALU.mult, op1=ALU.add)
                nc.vector.tensor_tensor(out=y_n, in0=y_n, in1=t_n, op=ALU.mult)
            nc.vector.tensor_copy(out=gstat[:, 1:2], in_=y_n)
            for ct in range(CT):
                ps_e = ps_big.tile([128, HW], F32, tag="psbig", name=f"pse_{b}_{ct}")
                nc.tensor.matmul(ps_e[:, 0:2], lhsT=em_sb[:, ct, :], rhs=gstat,
                                 start=True, stop=True)
                sc = smalls.tile([128, 1], F32, tag="sc", name=f"sc_{b}_{ct}")
                bi = smalls.tile([128, 1], F32, tag="bi", name=f"bi_{b}_{ct}")
                nc.vector.tensor_tensor(out=sc, in0=gng_sb[:, ct:ct + 1], in1=ps_e[:, 1:2], op=ALU.mult)
                nc.vector.tensor_tensor(out=bi, in0=ps_e[:, 0:1], in1=sc, op=ALU.mult)
                nc.vector.tensor_tensor(out=bi, in0=gnb_sb[:, ct:ct + 1], in1=bi, op=ALU.subtract)
                nc.vector.tensor_scalar(out=xns[ct], in0=xts[ct], scalar1=sc, scalar2=bi,
                                        op0=ALU.mult, op1=ALU.add)
            s["xts"], s["xns"] = xts, xns

        def emit_vt(b, sts):
            """vT[s, o] = sum_c xn[c, s] * WvT[c, o]; lhsT = xn tile ([c, s])."""
            s = state[b]
            vtas = s.setdefault("vtas", [None] * ST)
            for st in sts:
                ps_v = ps_big.tile([128, HW], F32, tag="psbig", name=f"psv_{b}_{st}")
                for kt in range(CT):
                    nc.tensor.matmul(ps_v[:, 0:C],
                                     lhsT=s["xns"][kt][:, st * 128:(st + 1) * 128],
                                     rhs=wq_sb[:, kt, 2 * C:3 * C],
                                     start=(kt == 0), stop=(kt == CT - 1))
                vta = vtapool.tile([128, NUM_HEADS, HD + 1], BF16, tag="vta",
                                   name=f"vta_{b}_{st}")
                nc.vector.memset(vta[:, :, HD:HD + 1], 1.0)
                nc.vector.tensor_tensor(
                    out=vta[:, :, 0:HD],
                    in0=ps_v[:, 0:C].rearrange("p (h d) -> p h d", h=NUM_HEADS),
                    in1=vbias_bc, op=ALU.add)
                vtas[st] = vta

        def emit_qk(b, ots):
            """q/k channel-major, in caller-chosen o-tile order."""
            s = state[b]
            qks = s.setdefault("qks", [None] * QKT)
            for ot in ots:
                ps_q = ps_big.tile([128, HW], F32, tag="psbig", name=f"psq_{b}_{ot}")
                for kt in range(CT):
                    for nh in range(NH):
                        nc.tensor.matmul(ps_q[:, nh * 512:(nh + 1) * 512],
                                         lhsT=wq_sb[:, kt, ot * 128:(ot + 1) * 128],
                                         rhs=s["xns"][kt][:, nh * 512:(nh + 1) * 512],
                                         start=(kt == 0), stop=(kt == CT - 1))
                qt = qkvpool.tile([128, HW], BF16, tag="qkv", name=f"qk_{b}_{ot}")
                nc.vector.tensor_scalar_add(out=qt, in0=ps_q, scalar1=qkvb_sb[:, ot:ot + 1])
                qks[ot] = qt

        def emit_pair(b, hp):
            """Head pair (2hp, 2hp+1): head 2hp on partitions 0:64, 2hp+1 on
            64:128; the two K=64 S-matmuls pack into disjoint PE row-groups
            via tile_position and run concurrently."""
            s = state[b]
            if "hts" not in s:
                s["hts"] = [hpool.tile([128, HW], BF16, tag="hm", name=f"hm_{b}_{i}")
                            for i in range(CT)]
            qt2 = s["qks"][hp]
            kt2 = s["qks"][CT + hp]
            vtas = s["vtas"]
            ps_os = [ps_av.tile([128, HW], F32, tag="psav", name=f"pso_{b}_{hp}_{j}")
                     for j in range(2)]
            ps_ss = {}

            def s_mm(st, j):
                p0 = j * 64
                t = ps_big.tile([128, HW], F32, tag="psbig",
                                name=f"pss_{b}_{hp}_{st}_{j}")
                for nh in range(NH):
                    nc.tensor.matmul(t[:, nh * 512:(nh + 1) * 512],
                                     lhsT=kt2[p0:p0 + 64, st * 128:(st + 1) * 128],
                                     rhs=qt2[p0:p0 + 64, nh * 512:(nh + 1) * 512],
                                     start=True, stop=True,
                                     tile_position=(p0, 0) if USE_TP else None)
                ps_ss[(st, j)] = t

            # software pipeline: stream j's S for round st+1 is emitted right
            # after its AV of round st, so ScalarE exps run back-to-back while
            # the PE computes the next round's logits. Only 2 S slots live.
            s_mm(0, 0)
            s_mm(0, 1)
            for st in range(ST):
                for j in range(2):
                    h = 2 * hp + j
                    pexp = exppool.tile([128, HW], BF16, tag="pexp",
                                        name=f"pexp_{b}_{hp}_{st}_{j}")
                    nc.scalar.activation(out=pexp, in_=ps_ss[(st, j)], func=ACTF.Exp,
                                         scale=1.0 / np.sqrt(HD))
                    for nh in range(NH):
                        nc.tensor.matmul(ps_os[j][0:HD + 1, nh * 512:(nh + 1) * 512],
                                         lhsT=vtas[st][:, h, :],
                                         rhs=pexp[:, nh * 512:(nh + 1) * 512],
                                         start=(st == 0), stop=(st == ST - 1))
                    if st + 1 < ST:
                        s_mm(st + 1, j)
            # normalize: rows 0..64 unnormalized, row 64 = rowsum r[t].
            # Copy out + reciprocal immediately (frees the PSUM accumulator),
            # then broadcast 1/r to 64 partitions via a DRAM-roundtrip DMA
            # (partition-broadcast APs are DRAM-only) and multiply on DVE.
            for j in range(2):
                h = 2 * hp + j
                ps_o = ps_os[j]
                # r is [1, 1024]: DVE reciprocal cost is free-size-bound, so
                # transpose it to [128, 8] via DMA first (reciprocal then runs
                # on all 128 lanes: ~60ns instead of ~7.8us), write back
                # t-flat to DRAM, broadcast to 64 partitions, multiply.
                r_sb = rpool.tile([1, HW], F32, tag="rinv", name=f"rsb_{b}_{h}")
                nc.vector.tensor_copy(out=r_sb, in_=ps_o[HD:HD + 1, :])
                hu = hupool.tile([HD, HW], F32, tag="hu", name=f"hu_{b}_{h}")
                nc.vector.tensor_copy(out=hu, in_=ps_o[0:HD, :])
                rs0 = drams.tile([1, HW], F32, tag="rs0", name=f"rs0_{b}_{h}")
                nc.sync.dma_start(out=rs0, in_=r_sb)
                rt = smalls.tile([128, 8], F32, tag="rt", name=f"rt_{b}_{h}")
                tr_ap = [[1, 128], [128, 8]]
                nc.sync.dma_start(
                    out=rt, in_=bass.AP(tensor=rs0.tensor, offset=rs0.offset, ap=tr_ap))
                nc.vector.reciprocal(out=rt, in_=rt)
                rs = drams.tile([1, HW], F32, tag="rs", name=f"rs_{b}_{h}")
                nc.sync.dma_start(
                    out=bass.AP(tensor=rs.tensor, offset=rs.offset, ap=tr_ap), in_=rt)
                rb = rbpool.tile([HD, HW], F32, tag="rb", name=f"rb_{b}_{h}")
                rs_bc = bass.AP(tensor=rs.tensor, offset=rs.offset,
                                ap=[[0, HD]] + list(rs.ap)[1:])
                nc.sync.dma_start(out=rb, in_=rs_bc)
                nc.vector.tensor_tensor(out=s["hts"][hp][j * 64:j * 64 + 64, :],
                                        in0=hu, in1=rb, op=ALU.mult)

        def emit_proj(b, ots):
            s = state[b]
            for ot in ots:
                ps_p = ps_big.tile([128, HW], F32, tag="psbig", name=f"psp_{b}_{ot}")
                for kt in range(CT):
                    for nh in range(NH):
                        nc.tensor.matmul(ps_p[:, nh * 512:(nh + 1) * 512],
                                         lhsT=wp_sb[:, kt, ot * 128:(ot + 1) * 128],
                                         rhs=s["hts"][kt][:, nh * 512:(nh + 1) * 512],
                                         start=(kt == 0), stop=(kt == CT - 1))
                yt = ypool.tile([128, HW], F32, tag="yt", name=f"yt_{b}_{ot}")
                nc.vector.tensor_scalar_add(out=yt, in0=ps_p, scalar1=projb_sb[:, ot:ot + 1])
                nc.vector.tensor_tensor(out=yt, in0=yt, in1=s["xts"][ot], op=ALU.add)
                nc.sync.dma_start(out=out_d[b, ot * 128:(ot + 1) * 128, :], in_=yt)

        # Interleaved emission: batch 1's prep rides inside batch 0's
        # (ACT-bound) attention phase so PE slack absorbs it.
        emit_gn(0)
        emit_weight_loads()
        emit_vt(0, range(ST))
        emit_qk(0, [0, 4])
        emit_pair(0, 0)
        emit_qk(0, [1, 5])
        emit_gn(1)
        emit_pair(0, 1)
        emit_qk(0, [2, 6])
        emit_vt(1, range(0, 4))
        emit_pair(0, 2)
        emit_qk(0, [3, 7])
        emit_vt(1, range(4, ST))
        emit_qk(1, [0, 4])
        emit_pair(0, 3)
        emit_qk(1, [1, 5])
        emit_pair(1, 0)
        emit_qk(1, [2, 6])
        emit_proj(0, [0, 1])
        emit_pair(1, 1)
        emit_qk(1, [3, 7])
        emit_proj(0, [2, 3])
        emit_pair(1, 2)
        emit_pair(1, 3)
        emit_proj(1, range(CT))


def make_host_inputs(x, gn_gamma, gn_beta, qkv_w, qkv_b, proj_w, proj_b):
    """Full inputs -> list of per-core in_maps."""
    x = np.asarray(x, dtype=np.float32).reshape(B, C, HW)
    wqkvT = np.ascontiguousarray(np.asarray(qkv_w, dtype=np.float32).T)
    wprojT = np.ascontiguousarray(np.asarray(proj_w, dtype=np.float32).T)
    gmat = np.zeros((CT, 128, NUM_GROUPS), dtype=np.float32)
    for t in range(CT):
        for k in range(128):
            gmat[t, k, (t * 128 + k) // GS] = 1.0
    emat = np.ascontiguousarray(gmat.transpose(0, 2, 1))
    shared = {
        "wqkvT": wqkvT.astype(ml_dtypes.bfloat16),
        "qkvb": np.asarray(qkv_b, dtype=np.float32),
        "wprojT": wprojT.astype(ml_dtypes.bfloat16),
        "projb": np.asarray(proj_b, dtype=np.float32),
        "gng": np.asarray(gn_gamma, dtype=np.float32),
        "gnb": np.asarray(gn_beta, dtype=np.float32),
        "gmat": gmat,
        "emat": emat,
    }
    return [dict(shared, x=np.ascontiguousarray(x[i * BPC:(i + 1) * BPC]))
            for i in range(NCORES)]


_NC_CACHE = {}


def _get_nc():
    if "nc" not in _NC_CACHE:
        _NC_CACHE["nc"] = build()
    return _NC_CACHE["nc"]


def kernel(x, gn_gamma, gn_beta, qkv_w, qkv_b, proj_w, proj_b):
    from concourse.bass_utils import run_bass_kernel_spmd
    nc = _get_nc()
    in_maps = make_host_inputs(x, gn_gamma, gn_beta, qkv_w, qkv_b, proj_w, proj_b)
    res = run_bass_kernel_spmd(nc, in_maps, list(range(NCORES)))
    out = np.concatenate([res.results[i]["out"] for i in range(NCORES)], axis=0)
    return out.reshape(B, C, H, W).astype(np.float32)

